# revision 23
# baseline (speedup 1.0000x reference)
"""ConvAttention kernel for 8x Trainium2 NeuronCores (Bass/Tile), v2.

Data-parallel over batch: B=32 -> 4 batches per core, one SPMD NEFF.

Math (per batch):
  k = conv1d(keys, kW1, pad=1) -> relu -> conv1d(kW2)      [100, 512]
  q = conv1d(queries, qW1, pad=1) -> relu -> conv1d(qW2) -> relu -> conv1d(qW3)  [100, 2048]
  s[t,c] = -0.0005*(q2[t] + k2[c] - 2*qk[t,c])   (augmented 128-row bf16 matmul)
  lp  = s - logsumexp_row(s) + logPu             (logPu = log(prior+1e-8), host)
  at  = e1*Pm / rowsum(e1*Pm)                    (e1 = exp(s), Pm = (prior+1e-8)*~mask, host)

Epilogue per 128x512 tile: one scalar Exp pass (e1), one vector reduce (s1),
one fused stt pass for lp ((s + -lse) + logPu), one fused stt pass for
em = e1*Pm with accum (s2), one cheap 4x-mode tensor_scalar for at = em*rr.
Attention of batch b-1 is interleaved into batch b's conv stream so the
tensor engine never stalls on PSUM epilogue drains.
"""

import numpy as np

import bass_rust
import concourse.bass as bass
import concourse.tile as tile
from concourse import mybir


def split_waits(nc, mm_keep=0, other_keep=1):
    """Hoist instruction-attached semaphore waits onto standalone
    InstEventSemaphore instructions (this walrus build rejects >1 attached
    wait per instruction and any wait on a 4-byte self-loading Matmult)."""
    for f in nc.m.functions:
        for bb in f.blocks:
            instrs = list(bb.instructions)
            new_instrs = []
            changed = False
            for ins in instrs:
                si = ins.sync_info
                waits = list(si.on_wait) if si is not None else []
                opc = type(ins).__name__
                if opc in ("InstMatmult", "InstMatmultMx"):
                    try:
                        is_2b = mybir.dt.size(ins.ins[0].dtype) == 2
                    except Exception:
                        is_2b = False
                    keep = 1 if is_2b else mm_keep
                else:
                    keep = other_keep
                if len(waits) > keep:
                    n_hoist = len(waits) - keep
                    for i in range(n_hoist):
                        nop = mybir.InstEventSemaphore(
                            name=f"{ins.name}-hw{i}", engine=ins.engine, ins=[], outs=[],
                            sync_info=bass_rust.SyncInfo(on_wait=[waits[i]], on_update=[]),
                        )
                        new_instrs.append(nop)
                    ins.sync_info = bass_rust.SyncInfo(
                        on_wait=waits[n_hoist:], on_update=list(si.on_update)
                    )
                    changed = True
                new_instrs.append(ins)
            if changed:
                bb.instructions = new_instrs
    return nc

F32 = mybir.dt.float32
BF16 = mybir.dt.bfloat16
FP8 = mybir.dt.float8e4
AF = mybir.ActivationFunctionType
OP = mybir.AluOpType

N_CORES = 8
B_FULL = 32
T1 = 2048
T2 = 512
CM = 100    # Cmel / Catt
CT = 512    # Ctext

_CACHE = {}


def build_program(B, split=True):
    """Build the per-core Bass program for B local batches."""
    nc = bass.Bass(trn_type="TRN2")

    queriesh = nc.dram_tensor("queriesh", [B, CM, T1], BF16, kind="ExternalInput")
    keysh8 = nc.dram_tensor("keysh8", [B, CT, T2], FP8, kind="ExternalInput")
    Pmh = nc.dram_tensor("Pmh", [B, T1, T2], BF16, kind="ExternalInput")
    logPuh = nc.dram_tensor("logPuh", [B, T1, T2], BF16, kind="ExternalInput")
    kW1dr = nc.dram_tensor("kW1dr", [128, 6, 2, 1024], FP8, kind="ExternalInput")
    kW2Th = nc.dram_tensor("kW2Th", [128, 8, CM], BF16, kind="ExternalInput")
    qW1Tp = nc.dram_tensor("qW1Tp", [128, 3, 200], BF16, kind="ExternalInput")
    qW2Tp = nc.dram_tensor("qW2Tp", [128, 2, CM], BF16, kind="ExternalInput")
    qW3Tp = nc.dram_tensor("qW3Tp", [128, CM], BF16, kind="ExternalInput")
    kb1r = nc.dram_tensor("kb1r", [128, 8], F32, kind="ExternalInput")
    kb2r = nc.dram_tensor("kb2r", [CM, 1], F32, kind="ExternalInput")
    qb1r = nc.dram_tensor("qb1r", [CM, 2], F32, kind="ExternalInput")
    qb2r = nc.dram_tensor("qb2r", [CM, 1], F32, kind="ExternalInput")
    qb3s = nc.dram_tensor("qb3s", [CM, 1], F32, kind="ExternalInput")
    augc = nc.dram_tensor("augc", [CM, 2], BF16, kind="ExternalInput")
    ones1 = nc.dram_tensor("ones1", [T1], BF16, kind="ExternalInput")
    zerosh = nc.dram_tensor("zerosh", [128, T1 + 2], BF16, kind="ExternalInput")

    attn_o = nc.dram_tensor("attn_o", [B, T1, T2], BF16, kind="ExternalOutput")
    lp_o = nc.dram_tensor("lp_o", [B, T1, T2], BF16, kind="ExternalOutput")

    NQ = T1 // 512   # 4 query chunks (also attention pairs 0-3 slots)
    NP = 8           # attention pairs per batch (each pair = 2 t-tiles of 128)

    from contextlib import ExitStack

    with ExitStack() as es:
        tc = es.enter_context(tile.TileContext(nc))
        pool = lambda name, bufs, **kw: es.enter_context(tc.tile_pool(name=name, bufs=bufs, **kw))
        wp = pool("wpool", 1)
        qpadp = pool("qpad", 2)
        kpadp = pool("kpad", 2)
        h1kp = pool("h1k", 9)
        h1qp = pool("h1q", 4)
        h2qp = pool("h2q", 2)
        lhsp = pool("lhs", 2)
        rhsp = pool("rhs", 2)
        sqp = pool("sq", 3)
        pmp = pool("pm", 2)
        lpup = pool("lpu", 2)
        e1p = pool("e1", 3)
        emp = pool("em", 3)
        aop = pool("ao", 2)
        lop = pool("lo", 2)
        smp = pool("small", 6)
        psA = pool("psA", 1, space="PSUM")
        psB = pool("psB", 1, space="PSUM")
        psAtt = pool("psAtt", 3, space="PSUM")

        # ---- persistent weights ----
        kw1_sb = wp.tile([128, 6, 2, 1024], FP8)
        nc.sync.dma_start(kw1_sb[:], kW1dr[:, :, :, :])
        kw2_sb = wp.tile([128, 8, CM], BF16)
        nc.sync.dma_start(kw2_sb[:], kW2Th[:, :, :])
        qw1_sb = wp.tile([128, 3, 200], BF16)
        nc.sync.dma_start(qw1_sb[:], qW1Tp[:, :, :])
        qw2_sb = wp.tile([128, 2, CM], BF16)
        nc.sync.dma_start(qw2_sb[:], qW2Tp[:, :, :])
        qw3_sb = wp.tile([128, CM], BF16)
        nc.sync.dma_start(qw3_sb[:], qW3Tp[:, :])
        kb1_sb = wp.tile([128, 8], F32)
        nc.sync.dma_start(kb1_sb[:], kb1r[:, :])
        kb2_sb = wp.tile([CM, 1], F32)
        nc.sync.dma_start(kb2_sb[:], kb2r[:, :])
        qb1_sb = wp.tile([CM, 2], F32)
        nc.sync.dma_start(qb1_sb[:], qb1r[:, :])
        qb2_sb = wp.tile([CM, 1], F32)
        nc.sync.dma_start(qb2_sb[:], qb2r[:, :])
        qb3s_sb = wp.tile([CM, 1], F32)
        nc.sync.dma_start(qb3s_sb[:], qb3s[:, :])
        augc_sb = wp.tile([CM, 2], BF16)
        nc.sync.dma_start(augc_sb[:], augc[:, :])
        neg5e4 = augc_sb[:, 0:1]
        neg500 = augc_sb[:, 1:2]

        # ---- one-time zero/one padding of rotating pool buffers ----
        # In-loop writes never touch these regions, so they persist.
        for _i in range(2):
            qp0 = qpadp.tile([128, T1 + 2], BF16, tag="qp")
            nc.gpsimd.dma_start(qp0[96:128, :], zerosh[96:128, :])
            nc.gpsimd.dma_start(qp0[0:CM, 0:1], zerosh[0:CM, 0:1])
            nc.gpsimd.dma_start(qp0[0:CM, T1 + 1 : T1 + 2], zerosh[0:CM, 0:1])
            kp0 = kpadp.tile([128, 12, T2], FP8, tag="kp")
            nc.vector.memset(kp0[:, 0:4, 0:1], 0.0)
            nc.vector.memset(kp0[:, 8:12, T2 - 1 : T2], 0.0)
            l0 = lhsp.tile([128, T1], BF16, tag="lhs")
            nc.gpsimd.dma_start(l0[96:128, :], zerosh[96:128, 0:T1])
            nc.sync.dma_start(l0[100:101, :], ones1[:].unsqueeze(0))
            r0 = rhsp.tile([128, T2], BF16, tag="rhs")
            nc.gpsimd.dma_start(r0[96:128, :], zerosh[96:128, 0:T2])
            h2q0 = h2qp.tile([128, 512], BF16, tag="h2q")
            nc.gpsimd.dma_start(h2q0[96:128, :], zerosh[96:128, 0:512])
        for _i in range(4):
            h1q0 = h1qp.tile([128, 512], BF16, tag="h1q")
            nc.gpsimd.dma_start(h1q0[96:128, :], zerosh[96:128, 0:512])

        # rotation trackers so interleaved emitters agree on buffers
        state = {}

        def queries_chunk(b, nq, qp, lhsT_att):
            """conv1q/conv2q/conv3q for 512 query cols + lhsT aug rows."""
            t0 = nq * 512
            h1q = []
            for ct in range(2):
                psq = psB.tile([128, 512], F32, tag="psB")
                for dk in range(3):
                    nc.tensor.matmul(
                        psq[0:CM, :],
                        qw1_sb[:, dk, ct * CM : (ct + 1) * CM],
                        qp[:, t0 + dk : t0 + dk + 512],
                        start=(dk == 0), stop=(dk == 2),
                    )
                h = h1qp.tile([128, 512], BF16, tag="h1q")
                if ct == 0:
                    nc.vector.tensor_scalar(
                        h[0:CM, :], psq[0:CM, :], qb1_sb[:, ct : ct + 1], 0.0,
                        op0=OP.add, op1=OP.max)
                else:
                    nc.scalar.activation(
                        h[0:CM, :], psq[0:CM, :], AF.Relu, bias=qb1_sb[:, ct : ct + 1])
                h1q.append(h)
            psq2 = psB.tile([128, 512], F32, tag="psB")
            nc.tensor.matmul(psq2[0:CM, :], qw2_sb[:, 0, :], h1q[0][:], start=True, stop=False)
            nc.tensor.matmul(psq2[0:CM, :], qw2_sb[:, 1, :], h1q[1][:], start=False, stop=True)
            h2q = h2qp.tile([128, 512], BF16, tag="h2q")
            nc.scalar.activation(h2q[0:CM, :], psq2[0:CM, :], AF.Relu, bias=qb2_sb[:, 0:1])
            psq3 = psB.tile([128, 512], F32, tag="psB")
            nc.tensor.matmul(psq3[0:CM, :], qw3_sb[:], h2q[:], start=True, stop=True)
            nc.scalar.activation(
                lhsT_att[0:CM, t0 : t0 + 512], psq3[0:CM, :], AF.Identity,
                bias=qb3s_sb[:, 0:1], scale=0.001,
            )
            # NOTE: no q^2 row — softmax/log_softmax are invariant to per-row
            # constants, so s' = 0.001*qk - 5e-4*k2 gives identical outputs.

        def keys_conv1_ct(b, ct, kp):
            psc = psA.tile([128, T2], F32, tag="psA")
            for i in range(6):
                nc.tensor.matmul(
                    psc[:],
                    kw1_sb[:, i, :, ct * 128 : (ct + 1) * 128],
                    kp[:, 2 * i : 2 * i + 2, :],
                    start=(i == 0), stop=(i == 5),
                    perf_mode=mybir.MatmulPerfMode.DoubleRow,
                )
            h1 = h1kp.tile([128, T2], BF16, tag="h1k")
            nc.scalar.activation(h1[:], psc[:], AF.Relu, bias=kb1_sb[:, ct : ct + 1])
            return h1

        def keys_finish(b, h1s, rhs_att):
            psk = psB.tile([128, T2], F32, tag="psB")
            for ct in range(8):
                nc.tensor.matmul(
                    psk[0:CM, :], kw2_sb[:, ct, :], h1s[ct][:],
                    start=(ct == 0), stop=(ct == 7),
                )
            nc.scalar.activation(rhs_att[0:CM, :], psk[0:CM, :], AF.Identity, bias=kb2_sb[:, 0:1])
            sqk = sqp.tile([CM, 512], BF16, tag="sq")
            nc.vector.tensor_tensor(sqk[:], rhs_att[0:CM, :], rhs_att[0:CM, :], op=OP.mult)
            psk2 = psB.tile([128, T2], F32, tag="psB")
            nc.tensor.matmul(psk2[0:1, :], neg5e4, sqk[:], start=True, stop=True)
            tmpk = sqp.tile([1, T2], BF16, tag="tmpk")
            nc.scalar.activation(tmpk[:], psk2[0:1, :], AF.Copy)
            nc.sync.dma_start(rhs_att[CM : CM + 1, :], tmpk[:])

        def stage_gq(b, gg):
            if ("att", b, gg) in state:
                return
            tg = gg * 512
            pm4 = pmp.tile([128, 4, T2], BF16, tag="pm")
            nc.sync.dma_start(
                pm4[:], Pmh[b, tg : tg + 512, :].rearrange("(j p) s -> p j s", p=128))
            lpu4 = lpup.tile([128, 4, T2], BF16, tag="lpu")
            nc.sync.dma_start(
                lpu4[:], logPuh[b, tg : tg + 512, :].rearrange("(j p) s -> p j s", p=128))
            at4 = aop.tile([128, 4, T2], BF16, tag="ao")
            lp4 = lop.tile([128, 4, T2], BF16, tag="lo")
            s24 = smp.tile([128, 4], F32, tag="s24")
            rr4 = smp.tile([128, 4], F32, tag="rr4")
            state[("att", b, gg)] = (pm4, lpu4, at4, lp4, s24, rr4)

        def attention_pair(b, p):
            """Pair p (t-tiles 2p, 2p+1) of batch b's attention + epilogue."""
            g = p // 2          # gq group (4 t-tiles)
            lhsT_att = state[("lhs", b)]
            rhs_att = state[("rhs", b)]
            stage_gq(b, g)
            if p % 2 == 1 and g + 2 <= 3:
                stage_gq(b, g + 2)
            pm4, lpu4, at4, lp4, s24, rr4 = state[("att", b, g)]

            psa2 = psAtt.tile([128, 2, T2], F32, tag="psAtt")
            for i in range(2):
                t0 = (2 * p + i) * 128
                nc.tensor.matmul(
                    psa2[:, i, :],
                    lhsT_att[:, t0 : t0 + 128],
                    rhs_att[:],
                    start=True, stop=True,
                )
            s1p = smp.tile([128, 2], F32, tag="s1p")
            e1s = []
            for i in range(2):
                e1j = e1p.tile([128, T2], BF16, tag="e1")
                nc.scalar.activation(e1j[:], psa2[:, i, :], AF.Exp,
                                     accum_out=s1p[:, i : i + 1])
                e1s.append(e1j)
            lse = smp.tile([128, 2], F32, tag="lse")
            nc.scalar.activation(lse[:], s1p[:], AF.Ln)
            for i in range(2):
                jj = (2 * p + i) % 4
                em = emp.tile([128, T2], BF16, tag="em")
                nc.vector.scalar_tensor_tensor(
                    em[:], e1s[i][:], 1.0, pm4[:, jj, :],
                    op0=OP.mult, op1=OP.mult, accum_out=s24[:, jj : jj + 1])
                nc.vector.scalar_tensor_tensor(
                    lp4[:, jj, :], psa2[:, i, :], lse[:, i : i + 1], lpu4[:, jj, :],
                    op0=OP.subtract, op1=OP.add)
                state[("em", b, jj)] = em
            jj0 = (2 * p) % 4
            nc.vector.reciprocal(rr4[:, jj0 : jj0 + 2], s24[:, jj0 : jj0 + 2])
            for i in range(2):
                jj = jj0 + i
                nc.vector.tensor_scalar(
                    at4[:, jj, :], state[("em", b, jj)][:], rr4[:, jj : jj + 1], None,
                    op0=OP.mult)
            if p % 2 == 1:
                tg = g * 512
                nc.sync.dma_start(
                    attn_o[b, tg : tg + 512, :].rearrange("(j p) s -> p j s", p=128), at4[:])
                nc.sync.dma_start(
                    lp_o[b, tg : tg + 512, :].rearrange("(j p) s -> p j s", p=128), lp4[:])

        for b in range(B):
            # ---- keys path first so attention(b) can pipeline with queries(b)
            qp = qpadp.tile([128, T1 + 2], BF16, tag="qp")
            nc.sync.dma_start(qp[0:CM, 1 : T1 + 1], queriesh[b, :, :])
            kp = kpadp.tile([128, 12, T2], FP8, tag="kp")
            nc.sync.dma_start(
                kp[:, 0:4, 1:T2],
                keysh8[b, :, 0 : T2 - 1].rearrange("(ch p) t -> p ch t", p=128))
            nc.sync.dma_start(
                kp[:, 4:8, :],
                keysh8[b, :, :].rearrange("(ch p) t -> p ch t", p=128))
            nc.sync.dma_start(
                kp[:, 8:12, 0 : T2 - 1],
                keysh8[b, :, 1:T2].rearrange("(ch p) t -> p ch t", p=128))
            rhs_att = rhsp.tile([128, T2], BF16, tag="rhs")
            state[("rhs", b)] = rhs_att
            stage_gq(b, 0)
            stage_gq(b, 1)
            h1s = []
            for ct in range(8):
                h1s.append(keys_conv1_ct(b, ct, kp))
            keys_finish(b, h1s, rhs_att)
            # ---- queries path + same-batch attention per 512-col chunk ----
            lhsT_att = lhsp.tile([128, T1], BF16, tag="lhs")
            state[("lhs", b)] = lhsT_att
            for nq in range(NQ):
                queries_chunk(b, nq, qp, lhsT_att)
                attention_pair(b, 2 * nq)
                attention_pair(b, 2 * nq + 1)

    nc.finalize()
    if split:
        split_waits(nc)
    return nc


def host_prep(inputs):
    """Host-side marshalling: weight transposes/padding, prior surfaces, shards."""
    q = np.ascontiguousarray(np.asarray(inputs["queries"], dtype=np.float32))
    k = np.ascontiguousarray(np.asarray(inputs["keys"], dtype=np.float32))
    prior = np.asarray(inputs["attn_prior"], dtype=np.float32)
    mask = np.asarray(inputs["mask"])
    kW1 = np.asarray(inputs["kW1"], dtype=np.float32)
    kb1 = np.asarray(inputs["kb1"], dtype=np.float32)
    kW2 = np.asarray(inputs["kW2"], dtype=np.float32)
    kb2 = np.asarray(inputs["kb2"], dtype=np.float32)
    qW1 = np.asarray(inputs["qW1"], dtype=np.float32)
    qb1 = np.asarray(inputs["qb1"], dtype=np.float32)
    qW2 = np.asarray(inputs["qW2"], dtype=np.float32)
    qb2 = np.asarray(inputs["qb2"], dtype=np.float32)
    qW3 = np.asarray(inputs["qW3"], dtype=np.float32)
    qb3 = np.asarray(inputs["qb3"], dtype=np.float32)

    import ml_dtypes
    bf16 = ml_dtypes.bfloat16

    def padp(a, p=128):
        out = np.zeros((p,) + a.shape[1:], a.dtype)
        out[: a.shape[0]] = a
        return out

    fp8 = ml_dtypes.float8_e4m3
    kW2T = np.ascontiguousarray(kW2[:, :, 0].T) / np.float32(16.0)   # [1024, 100]
    kW2Th = np.ascontiguousarray(
        kW2T.reshape(8, 128, CM).transpose(1, 0, 2)).astype(bf16)  # [128, 8, 100]
    w16 = kW1 * np.float32(16.0)                          # [1024, 512, 3]
    chunks = np.stack([
        np.ascontiguousarray(w16[:, (c % 4) * 128 : (c % 4 + 1) * 128, c // 4].T)
        for c in range(12)])                              # [12, 128, 1024]
    kW1drh = np.ascontiguousarray(
        chunks.reshape(6, 2, 128, 1024).transpose(2, 0, 1, 3)).astype(fp8)
    qW2T = np.ascontiguousarray(qW2[:, :, 0].T)          # [200, 100]
    qW2Tp = padp(np.ascontiguousarray(
        qW2T.reshape(2, CM, CM).transpose(1, 0, 2)).reshape(CM, 2 * CM)).reshape(
        128, 2, CM).astype(bf16)
    shared = {
        "kW1dr": kW1drh,
        "kW2Th": kW2Th,
        "qW1Tp": padp(np.ascontiguousarray(qW1.transpose(1, 2, 0))
                      .reshape(CM, 3 * 200)).reshape(128, 3, 200).astype(bf16),
        "qW2Tp": qW2Tp,
        "qW3Tp": padp(np.ascontiguousarray(qW3[:, :, 0].T)).astype(bf16),
        "kb1r": np.ascontiguousarray(16.0 * kb1.reshape(8, 128).T).astype(np.float32),
        "kb2r": np.ascontiguousarray(kb2[:, None]),
        "qb1r": np.ascontiguousarray(qb1.reshape(2, CM).T),
        "qb2r": np.ascontiguousarray(qb2[:, None]),
        "qb3s": np.ascontiguousarray(0.001 * qb3[:, None]),
        "augc": np.ascontiguousarray(
            np.stack([np.full(CM, -0.0005, np.float32),
                      np.full(CM, -500.0, np.float32)], axis=1)).astype(bf16),
        "ones1": np.ones(T1, np.float32).astype(bf16),
        "zerosh": np.zeros((128, T1 + 2), np.float32).astype(bf16),
    }
    prior_eps = prior + np.float32(1e-8)
    logPu = np.log(prior_eps)
    m01 = (~mask[:, :, 0]).astype(np.float32)            # [B, T2], 1 = keep
    Pm = prior_eps * m01[:, None, :]

    Bl = B_FULL // N_CORES
    in_maps = []
    for c in range(N_CORES):
        sl = slice(c * Bl, (c + 1) * Bl)
        in_maps.append({
            "queriesh": np.ascontiguousarray(q[sl]).astype(bf16),
            "keysh8": np.clip(np.ascontiguousarray(k[sl]), -240, 240).astype(fp8),
            "Pmh": np.ascontiguousarray(Pm[sl]).astype(bf16),
            "logPuh": np.ascontiguousarray(logPu[sl]).astype(bf16),
            **shared,
        })
    return in_maps


def _get_exec():
    """Compile the SPMD executable (8 cores, shard_map over axis 0)."""
    if "exec" in _CACHE:
        return _CACHE["exec"]
    import jax
    from jax.sharding import Mesh, PartitionSpec, NamedSharding
    from jax.experimental.shard_map import shard_map
    from concourse import bass2jax

    Bl = B_FULL // N_CORES
    nc = build_program(Bl)
    bass2jax.install_neuronx_cc_hook()

    partition_name = nc.partition_id_tensor.name if nc.partition_id_tensor else None
    in_names, out_names, out_avals, zero_shapes = [], [], [], []
    for alloc in nc.m.functions[0].allocations:
        if not isinstance(alloc, mybir.MemoryLocationSet):
            continue
        name = alloc.memorylocations[0].name
        if alloc.kind == "ExternalInput":
            if name != partition_name:
                in_names.append(name)
        elif alloc.kind == "ExternalOutput":
            np_dtype = mybir.dt.np(alloc.dtype)
            out_avals.append(jax.core.ShapedArray(tuple(alloc.tensor_shape), np_dtype))
            out_names.append(name)
            zero_shapes.append((tuple(alloc.tensor_shape), np_dtype))
    n_params = len(in_names)
    all_names = in_names + out_names
    if partition_name is not None:
        all_names.append(partition_name)

    def _body(*args):
        operands = list(args)
        if partition_name is not None:
            operands.append(bass2jax.partition_id_tensor())
        outs = bass2jax._bass_exec_p.bind(
            *operands,
            out_avals=tuple(out_avals),
            in_names=tuple(all_names),
            out_names=tuple(out_names),
            lowering_input_output_aliases=(),
            sim_require_finite=True,
            sim_require_nnan=True,
            nc=nc,
        )
        return tuple(outs)

    devices = jax.devices()[:N_CORES]
    mesh = Mesh(np.asarray(devices), ("core",))
    spec = PartitionSpec("core")
    sharded = jax.jit(
        shard_map(
            _body,
            mesh=mesh,
            in_specs=(spec,) * (n_params + len(out_names)),
            out_specs=(spec,) * len(out_names),
            check_rep=False,
        ),
        keep_unused=True,
    )
    sharding = NamedSharding(mesh, spec)
    _CACHE["exec"] = dict(
        nc=nc, fn=sharded, in_names=in_names, out_names=out_names,
        zero_shapes=zero_shapes, sharding=sharding,
    )
    return _CACHE["exec"]


def _device_args(in_maps):
    """Concat per-core input maps along axis 0 and device_put with sharding."""
    import jax
    ex = _get_exec()
    args = []
    for name in ex["in_names"]:
        arr = np.concatenate([m[name] for m in in_maps], axis=0)
        args.append(arr)
    for shape, dt in ex["zero_shapes"]:
        args.append(np.zeros((N_CORES * shape[0],) + shape[1:], dt))
    return [jax.device_put(a, ex["sharding"]) for a in args]


def kernel(**inputs):
    ex = _get_exec()
    in_maps = host_prep(inputs)
    dargs = _device_args(in_maps)
    outs = ex["fn"](*dargs)
    attn = np.asarray(outs[ex["out_names"].index("attn_o")]).astype(np.float32)
    lp = np.asarray(outs[ex["out_names"].index("lp_o")]).astype(np.float32)
    attn = attn.reshape(B_FULL, 1, T1, T2)
    lp = lp.reshape(B_FULL, 1, T1, T2)
    return attn, lp


def bench(inputs, warmup=2, n_small=16, n_big=64):
    """Marginal per-execution time: (t(n_big) - t(n_small)) / (n_big - n_small),
    which cancels the fixed axon dispatch overhead."""
    import time
    import jax
    ex = _get_exec()
    in_maps = host_prep(inputs)
    dargs = _device_args(in_maps)
    for _ in range(warmup):
        jax.block_until_ready(ex["fn"](*dargs))
    t0 = time.perf_counter()
    out = ex["fn"](*dargs)
    jax.block_until_ready(out)
    t_single = time.perf_counter() - t0

    def burst(n):
        t0 = time.perf_counter()
        outs = [ex["fn"](*dargs) for _ in range(n)]
        jax.block_until_ready(outs)
        return time.perf_counter() - t0

    burst(4)
    margs = []
    for _ in range(3):
        ts = burst(n_small)
        tb = burst(n_big)
        margs.append((tb - ts) / (n_big - n_small))
    t_marg = min(margs)
    return t_single, t_marg


# revision 24
# speedup vs baseline: 1.6676x; 1.6676x over previous
"""ConvAttention kernel for 8x Trainium2 NeuronCores (Bass/Tile), v2.

Data-parallel over batch: B=32 -> 4 batches per core, one SPMD NEFF.

Math (per batch):
  k = conv1d(keys, kW1, pad=1) -> relu -> conv1d(kW2)      [100, 512]
  q = conv1d(queries, qW1, pad=1) -> relu -> conv1d(qW2) -> relu -> conv1d(qW3)  [100, 2048]
  s[t,c] = -0.0005*(q2[t] + k2[c] - 2*qk[t,c])   (augmented 128-row bf16 matmul)
  lp  = s - logsumexp_row(s) + logPu             (logPu = log(prior+1e-8), host)
  at  = e1*Pm / rowsum(e1*Pm)                    (e1 = exp(s), Pm = (prior+1e-8)*~mask, host)

Epilogue per 128x512 tile: one scalar Exp pass (e1), one vector reduce (s1),
one fused stt pass for lp ((s + -lse) + logPu), one fused stt pass for
em = e1*Pm with accum (s2), one cheap 4x-mode tensor_scalar for at = em*rr.
Attention of batch b-1 is interleaved into batch b's conv stream so the
tensor engine never stalls on PSUM epilogue drains.
"""

import numpy as np

import bass_rust
import concourse.bass as bass
import concourse.tile as tile
from concourse import mybir


def split_waits(nc, mm_keep=0, other_keep=1):
    """Hoist instruction-attached semaphore waits onto standalone
    InstEventSemaphore instructions (this walrus build rejects >1 attached
    wait per instruction and any wait on a 4-byte self-loading Matmult)."""
    for f in nc.m.functions:
        for bb in f.blocks:
            instrs = list(bb.instructions)
            new_instrs = []
            changed = False
            for ins in instrs:
                si = ins.sync_info
                waits = list(si.on_wait) if si is not None else []
                opc = type(ins).__name__
                if opc in ("InstMatmult", "InstMatmultMx"):
                    try:
                        is_2b = mybir.dt.size(ins.ins[0].dtype) == 2
                    except Exception:
                        is_2b = False
                    keep = 1 if is_2b else mm_keep
                else:
                    keep = other_keep
                if len(waits) > keep:
                    n_hoist = len(waits) - keep
                    for i in range(n_hoist):
                        nop = mybir.InstEventSemaphore(
                            name=f"{ins.name}-hw{i}", engine=ins.engine, ins=[], outs=[],
                            sync_info=bass_rust.SyncInfo(on_wait=[waits[i]], on_update=[]),
                        )
                        new_instrs.append(nop)
                    ins.sync_info = bass_rust.SyncInfo(
                        on_wait=waits[n_hoist:], on_update=list(si.on_update)
                    )
                    changed = True
                new_instrs.append(ins)
            if changed:
                bb.instructions = new_instrs
    return nc

F32 = mybir.dt.float32
BF16 = mybir.dt.bfloat16
FP8 = mybir.dt.float8e4
AF = mybir.ActivationFunctionType
OP = mybir.AluOpType

N_CORES = 8
B_FULL = 32
T1 = 2048
T2 = 512
CM = 100    # Cmel / Catt
CT = 512    # Ctext

_CACHE = {}


def build_program(B, split=True):
    """Build the per-core Bass program for B local batches."""
    nc = bass.Bass(trn_type="TRN2")

    queriesh = nc.dram_tensor("queriesh", [B, CM, T1], BF16, kind="ExternalInput")
    keysh8 = nc.dram_tensor("keysh8", [B, CT, T2], FP8, kind="ExternalInput")
    Pmh = nc.dram_tensor("Pmh", [B, T1, T2], BF16, kind="ExternalInput")
    logPuh = nc.dram_tensor("logPuh", [B, T1, T2], BF16, kind="ExternalInput")
    kW1dr = nc.dram_tensor("kW1dr", [128, 6, 2, 1024], FP8, kind="ExternalInput")
    kW2Th = nc.dram_tensor("kW2Th", [128, 8, CM], BF16, kind="ExternalInput")
    qW1Tp = nc.dram_tensor("qW1Tp", [128, 3, 200], BF16, kind="ExternalInput")
    qW2Tp = nc.dram_tensor("qW2Tp", [128, 2, CM], BF16, kind="ExternalInput")
    qW3Tp = nc.dram_tensor("qW3Tp", [128, CM], BF16, kind="ExternalInput")
    kb1r = nc.dram_tensor("kb1r", [128, 8], F32, kind="ExternalInput")
    kb2r = nc.dram_tensor("kb2r", [CM, 1], F32, kind="ExternalInput")
    qb1r = nc.dram_tensor("qb1r", [CM, 2], F32, kind="ExternalInput")
    qb2r = nc.dram_tensor("qb2r", [CM, 1], F32, kind="ExternalInput")
    qb3s = nc.dram_tensor("qb3s", [CM, 1], F32, kind="ExternalInput")
    augc = nc.dram_tensor("augc", [CM, 2], BF16, kind="ExternalInput")
    ones1 = nc.dram_tensor("ones1", [T1], BF16, kind="ExternalInput")
    zerosh = nc.dram_tensor("zerosh", [128, T1 + 2], BF16, kind="ExternalInput")

    attn_o = nc.dram_tensor("attn_o", [B, T1, T2], BF16, kind="ExternalOutput")
    lp_o = nc.dram_tensor("lp_o", [B, T1, T2], BF16, kind="ExternalOutput")

    NQ = T1 // 512   # 4 query chunks (also attention pairs 0-3 slots)
    NP = 8           # attention pairs per batch (each pair = 2 t-tiles of 128)

    from contextlib import ExitStack

    with ExitStack() as es:
        tc = es.enter_context(tile.TileContext(nc))
        pool = lambda name, bufs, **kw: es.enter_context(tc.tile_pool(name=name, bufs=bufs, **kw))
        wp = pool("wpool", 1)
        qpadp = pool("qpad", 2)
        kpadp = pool("kpad", 2)
        h1kp = pool("h1k", 9)
        h1qp = pool("h1q", 4)
        h2qp = pool("h2q", 2)
        lhsp = pool("lhs", 2)
        rhsp = pool("rhs", 2)
        sqp = pool("sq", 3)
        pmp = pool("pm", 2)
        lpup = pool("lpu", 2)
        e1p = pool("e1", 3)
        emp = pool("em", 3)
        aop = pool("ao", 2)
        lop = pool("lo", 2)
        smp = pool("small", 6)
        psA = pool("psA", 2, space="PSUM")
        psB = pool("psB", 2, space="PSUM")
        psAtt = pool("psAtt", 2, space="PSUM")

        # ---- persistent weights ----
        kw1_sb = wp.tile([128, 6, 2, 1024], FP8)
        nc.sync.dma_start(kw1_sb[:], kW1dr[:, :, :, :])
        kw2_sb = wp.tile([128, 8, CM], BF16)
        nc.sync.dma_start(kw2_sb[:], kW2Th[:, :, :])
        qw1_sb = wp.tile([128, 3, 200], BF16)
        nc.sync.dma_start(qw1_sb[:], qW1Tp[:, :, :])
        qw2_sb = wp.tile([128, 2, CM], BF16)
        nc.sync.dma_start(qw2_sb[:], qW2Tp[:, :, :])
        qw3_sb = wp.tile([128, CM], BF16)
        nc.sync.dma_start(qw3_sb[:], qW3Tp[:, :])
        kb1_sb = wp.tile([128, 8], F32)
        nc.sync.dma_start(kb1_sb[:], kb1r[:, :])
        kb2_sb = wp.tile([CM, 1], F32)
        nc.sync.dma_start(kb2_sb[:], kb2r[:, :])
        qb1_sb = wp.tile([CM, 2], F32)
        nc.sync.dma_start(qb1_sb[:], qb1r[:, :])
        qb2_sb = wp.tile([CM, 1], F32)
        nc.sync.dma_start(qb2_sb[:], qb2r[:, :])
        qb3s_sb = wp.tile([CM, 1], F32)
        nc.sync.dma_start(qb3s_sb[:], qb3s[:, :])
        augc_sb = wp.tile([CM, 2], BF16)
        nc.sync.dma_start(augc_sb[:], augc[:, :])
        neg5e4 = augc_sb[:, 0:1]
        neg500 = augc_sb[:, 1:2]

        # ---- one-time zero/one padding of rotating pool buffers ----
        # In-loop writes never touch these regions, so they persist.
        for _i in range(2):
            qp0 = qpadp.tile([128, T1 + 2], BF16, tag="qp")
            nc.gpsimd.dma_start(qp0[96:128, :], zerosh[96:128, :])
            nc.gpsimd.dma_start(qp0[0:CM, 0:1], zerosh[0:CM, 0:1])
            nc.gpsimd.dma_start(qp0[0:CM, T1 + 1 : T1 + 2], zerosh[0:CM, 0:1])
            kp0 = kpadp.tile([128, 12, T2], FP8, tag="kp")
            nc.vector.memset(kp0[:, 0:4, 0:1], 0.0)
            nc.vector.memset(kp0[:, 8:12, T2 - 1 : T2], 0.0)
            l0 = lhsp.tile([128, T1], BF16, tag="lhs")
            nc.gpsimd.dma_start(l0[96:128, :], zerosh[96:128, 0:T1])
            nc.sync.dma_start(l0[100:101, :], ones1[:].unsqueeze(0))
            r0 = rhsp.tile([128, T2], BF16, tag="rhs")
            nc.gpsimd.dma_start(r0[96:128, :], zerosh[96:128, 0:T2])
            h2q0 = h2qp.tile([128, 512], BF16, tag="h2q")
            nc.gpsimd.dma_start(h2q0[96:128, :], zerosh[96:128, 0:512])
        for _i in range(4):
            h1q0 = h1qp.tile([128, 512], BF16, tag="h1q")
            nc.gpsimd.dma_start(h1q0[96:128, :], zerosh[96:128, 0:512])

        # rotation trackers so interleaved emitters agree on buffers
        state = {}

        def queries_chunk(b, nq, qp, lhsT_att):
            """conv1q/conv2q/conv3q for 512 query cols + lhsT aug rows."""
            t0 = nq * 512
            h1q = []
            for ct in range(2):
                psq = psB.tile([128, 512], F32, tag="psB")
                for dk in range(3):
                    nc.tensor.matmul(
                        psq[0:CM, :],
                        qw1_sb[:, dk, ct * CM : (ct + 1) * CM],
                        qp[:, t0 + dk : t0 + dk + 512],
                        start=(dk == 0), stop=(dk == 2),
                    )
                h = h1qp.tile([128, 512], BF16, tag="h1q")
                if ct == 0:
                    nc.vector.tensor_scalar(
                        h[0:CM, :], psq[0:CM, :], qb1_sb[:, ct : ct + 1], 0.0,
                        op0=OP.add, op1=OP.max)
                else:
                    nc.scalar.activation(
                        h[0:CM, :], psq[0:CM, :], AF.Relu, bias=qb1_sb[:, ct : ct + 1])
                h1q.append(h)
            psq2 = psB.tile([128, 512], F32, tag="psB")
            nc.tensor.matmul(psq2[0:CM, :], qw2_sb[:, 0, :], h1q[0][:], start=True, stop=False)
            nc.tensor.matmul(psq2[0:CM, :], qw2_sb[:, 1, :], h1q[1][:], start=False, stop=True)
            h2q = h2qp.tile([128, 512], BF16, tag="h2q")
            nc.scalar.activation(h2q[0:CM, :], psq2[0:CM, :], AF.Relu, bias=qb2_sb[:, 0:1])
            psq3 = psB.tile([128, 512], F32, tag="psB")
            nc.tensor.matmul(psq3[0:CM, :], qw3_sb[:], h2q[:], start=True, stop=True)
            nc.scalar.activation(
                lhsT_att[0:CM, t0 : t0 + 512], psq3[0:CM, :], AF.Identity,
                bias=qb3s_sb[:, 0:1], scale=0.001,
            )
            # NOTE: no q^2 row — softmax/log_softmax are invariant to per-row
            # constants, so s' = 0.001*qk - 5e-4*k2 gives identical outputs.

        def keys_conv1_ct(b, ct, kp):
            psc = psA.tile([128, T2], F32, tag="psA")
            for i in range(6):
                nc.tensor.matmul(
                    psc[:],
                    kw1_sb[:, i, :, ct * 128 : (ct + 1) * 128],
                    kp[:, 2 * i : 2 * i + 2, :],
                    start=(i == 0), stop=(i == 5),
                    perf_mode=mybir.MatmulPerfMode.DoubleRow,
                )
            h1 = h1kp.tile([128, T2], BF16, tag="h1k")
            nc.scalar.activation(h1[:], psc[:], AF.Relu, bias=kb1_sb[:, ct : ct + 1])
            return h1

        def keys_finish(b, h1s, rhs_att):
            psk = psB.tile([128, T2], F32, tag="psB")
            for ct in range(8):
                nc.tensor.matmul(
                    psk[0:CM, :], kw2_sb[:, ct, :], h1s[ct][:],
                    start=(ct == 0), stop=(ct == 7),
                )
            nc.scalar.activation(rhs_att[0:CM, :], psk[0:CM, :], AF.Identity, bias=kb2_sb[:, 0:1])
            sqk = sqp.tile([CM, 512], BF16, tag="sq")
            nc.vector.tensor_tensor(sqk[:], rhs_att[0:CM, :], rhs_att[0:CM, :], op=OP.mult)
            psk2 = psB.tile([128, T2], F32, tag="psB")
            nc.tensor.matmul(psk2[0:1, :], neg5e4, sqk[:], start=True, stop=True)
            tmpk = sqp.tile([1, T2], BF16, tag="tmpk")
            nc.scalar.activation(tmpk[:], psk2[0:1, :], AF.Copy)
            nc.sync.dma_start(rhs_att[CM : CM + 1, :], tmpk[:])

        def attention_pair(b, p):
            """Pair p (t-tiles 2p, 2p+1) of batch b's attention + epilogue."""
            g = p // 2          # gq group (4 t-tiles)
            lhsT_att = state[("lhs", b)]
            rhs_att = state[("rhs", b)]
            if p % 2 == 0:
                tg = g * 512
                pm4 = pmp.tile([128, 4, T2], BF16, tag="pm")
                nc.sync.dma_start(
                    pm4[:], Pmh[b, tg : tg + 512, :].rearrange("(j p) s -> p j s", p=128))
                lpu4 = lpup.tile([128, 4, T2], BF16, tag="lpu")
                nc.sync.dma_start(
                    lpu4[:], logPuh[b, tg : tg + 512, :].rearrange("(j p) s -> p j s", p=128))
                at4 = aop.tile([128, 4, T2], BF16, tag="ao")
                lp4 = lop.tile([128, 4, T2], BF16, tag="lo")
                s24 = smp.tile([128, 4], F32, tag="s24")
                rr4 = smp.tile([128, 4], F32, tag="rr4")
                state[("att", b, g)] = (pm4, lpu4, at4, lp4, s24, rr4)
            pm4, lpu4, at4, lp4, s24, rr4 = state[("att", b, g)]

            psa2 = psAtt.tile([128, 2, T2], F32, tag="psAtt")
            for i in range(2):
                t0 = (2 * p + i) * 128
                nc.tensor.matmul(
                    psa2[:, i, :],
                    lhsT_att[:, t0 : t0 + 128],
                    rhs_att[:],
                    start=True, stop=True,
                )
            s1p = smp.tile([128, 2], F32, tag="s1p")
            e1s = []
            for i in range(2):
                e1j = e1p.tile([128, T2], BF16, tag="e1")
                nc.scalar.activation(e1j[:], psa2[:, i, :], AF.Exp,
                                     accum_out=s1p[:, i : i + 1])
                e1s.append(e1j)
            lse = smp.tile([128, 2], F32, tag="lse")
            nc.scalar.activation(lse[:], s1p[:], AF.Ln)
            for i in range(2):
                jj = (2 * p + i) % 4
                em = emp.tile([128, T2], BF16, tag="em")
                nc.vector.scalar_tensor_tensor(
                    em[:], e1s[i][:], 1.0, pm4[:, jj, :],
                    op0=OP.mult, op1=OP.mult, accum_out=s24[:, jj : jj + 1])
                nc.vector.scalar_tensor_tensor(
                    lp4[:, jj, :], psa2[:, i, :], lse[:, i : i + 1], lpu4[:, jj, :],
                    op0=OP.subtract, op1=OP.add)
                state[("em", b, jj)] = em
            jj0 = (2 * p) % 4
            nc.vector.reciprocal(rr4[:, jj0 : jj0 + 2], s24[:, jj0 : jj0 + 2])
            for i in range(2):
                jj = jj0 + i
                nc.vector.tensor_scalar(
                    at4[:, jj, :], state[("em", b, jj)][:], rr4[:, jj : jj + 1], None,
                    op0=OP.mult)
            if p % 2 == 1:
                tg = g * 512
                nc.sync.dma_start(
                    attn_o[b, tg : tg + 512, :].rearrange("(j p) s -> p j s", p=128), at4[:])
                nc.sync.dma_start(
                    lp_o[b, tg : tg + 512, :].rearrange("(j p) s -> p j s", p=128), lp4[:])

        for b in range(B):
            # ---- queries path (+ interleave attention pairs 0-3 of b-1) ----
            qp = qpadp.tile([128, T1 + 2], BF16, tag="qp")
            nc.sync.dma_start(qp[0:CM, 1 : T1 + 1], queriesh[b, :, :])
            lhsT_att = lhsp.tile([128, T1], BF16, tag="lhs")
            state[("lhs", b)] = lhsT_att
            for nq in range(NQ):
                queries_chunk(b, nq, qp, lhsT_att)
                if b >= 1:
                    attention_pair(b - 1, nq)
            # ---- keys path (+ interleave attention pairs 4-7 of b-1) ----
            kp = kpadp.tile([128, 12, T2], FP8, tag="kp")
            nc.sync.dma_start(
                kp[:, 0:4, 1:T2],
                keysh8[b, :, 0 : T2 - 1].rearrange("(ch p) t -> p ch t", p=128))
            nc.sync.dma_start(
                kp[:, 4:8, :],
                keysh8[b, :, :].rearrange("(ch p) t -> p ch t", p=128))
            nc.sync.dma_start(
                kp[:, 8:12, 0 : T2 - 1],
                keysh8[b, :, 1:T2].rearrange("(ch p) t -> p ch t", p=128))
            rhs_att = rhsp.tile([128, T2], BF16, tag="rhs")
            state[("rhs", b)] = rhs_att
            h1s = []
            for ct in range(8):
                h1s.append(keys_conv1_ct(b, ct, kp))
                if b >= 1 and ct % 2 == 0:
                    attention_pair(b - 1, 4 + ct // 2)
            keys_finish(b, h1s, rhs_att)
        for p in range(NP):
            attention_pair(B - 1, p)

    nc.finalize()
    if split:
        split_waits(nc)
    return nc


def host_prep(inputs):
    """Host-side marshalling: weight transposes/padding, prior surfaces, shards."""
    q = np.ascontiguousarray(np.asarray(inputs["queries"], dtype=np.float32))
    k = np.ascontiguousarray(np.asarray(inputs["keys"], dtype=np.float32))
    prior = np.asarray(inputs["attn_prior"], dtype=np.float32)
    mask = np.asarray(inputs["mask"])
    kW1 = np.asarray(inputs["kW1"], dtype=np.float32)
    kb1 = np.asarray(inputs["kb1"], dtype=np.float32)
    kW2 = np.asarray(inputs["kW2"], dtype=np.float32)
    kb2 = np.asarray(inputs["kb2"], dtype=np.float32)
    qW1 = np.asarray(inputs["qW1"], dtype=np.float32)
    qb1 = np.asarray(inputs["qb1"], dtype=np.float32)
    qW2 = np.asarray(inputs["qW2"], dtype=np.float32)
    qb2 = np.asarray(inputs["qb2"], dtype=np.float32)
    qW3 = np.asarray(inputs["qW3"], dtype=np.float32)
    qb3 = np.asarray(inputs["qb3"], dtype=np.float32)

    import ml_dtypes
    bf16 = ml_dtypes.bfloat16

    def padp(a, p=128):
        out = np.zeros((p,) + a.shape[1:], a.dtype)
        out[: a.shape[0]] = a
        return out

    fp8 = ml_dtypes.float8_e4m3
    kW2T = np.ascontiguousarray(kW2[:, :, 0].T) / np.float32(16.0)   # [1024, 100]
    kW2Th = np.ascontiguousarray(
        kW2T.reshape(8, 128, CM).transpose(1, 0, 2)).astype(bf16)  # [128, 8, 100]
    w16 = kW1 * np.float32(16.0)                          # [1024, 512, 3]
    chunks = np.stack([
        np.ascontiguousarray(w16[:, (c % 4) * 128 : (c % 4 + 1) * 128, c // 4].T)
        for c in range(12)])                              # [12, 128, 1024]
    kW1drh = np.ascontiguousarray(
        chunks.reshape(6, 2, 128, 1024).transpose(2, 0, 1, 3)).astype(fp8)
    qW2T = np.ascontiguousarray(qW2[:, :, 0].T)          # [200, 100]
    qW2Tp = padp(np.ascontiguousarray(
        qW2T.reshape(2, CM, CM).transpose(1, 0, 2)).reshape(CM, 2 * CM)).reshape(
        128, 2, CM).astype(bf16)
    shared = {
        "kW1dr": kW1drh,
        "kW2Th": kW2Th,
        "qW1Tp": padp(np.ascontiguousarray(qW1.transpose(1, 2, 0))
                      .reshape(CM, 3 * 200)).reshape(128, 3, 200).astype(bf16),
        "qW2Tp": qW2Tp,
        "qW3Tp": padp(np.ascontiguousarray(qW3[:, :, 0].T)).astype(bf16),
        "kb1r": np.ascontiguousarray(16.0 * kb1.reshape(8, 128).T).astype(np.float32),
        "kb2r": np.ascontiguousarray(kb2[:, None]),
        "qb1r": np.ascontiguousarray(qb1.reshape(2, CM).T),
        "qb2r": np.ascontiguousarray(qb2[:, None]),
        "qb3s": np.ascontiguousarray(0.001 * qb3[:, None]),
        "augc": np.ascontiguousarray(
            np.stack([np.full(CM, -0.0005, np.float32),
                      np.full(CM, -500.0, np.float32)], axis=1)).astype(bf16),
        "ones1": np.ones(T1, np.float32).astype(bf16),
        "zerosh": np.zeros((128, T1 + 2), np.float32).astype(bf16),
    }
    prior_eps = prior + np.float32(1e-8)
    logPu = np.log(prior_eps)
    m01 = (~mask[:, :, 0]).astype(np.float32)            # [B, T2], 1 = keep
    Pm = prior_eps * m01[:, None, :]

    Bl = B_FULL // N_CORES
    in_maps = []
    for c in range(N_CORES):
        sl = slice(c * Bl, (c + 1) * Bl)
        in_maps.append({
            "queriesh": np.ascontiguousarray(q[sl]).astype(bf16),
            "keysh8": np.clip(np.ascontiguousarray(k[sl]), -240, 240).astype(fp8),
            "Pmh": np.ascontiguousarray(Pm[sl]).astype(bf16),
            "logPuh": np.ascontiguousarray(logPu[sl]).astype(bf16),
            **shared,
        })
    return in_maps


def _get_exec():
    """Compile the SPMD executable (8 cores, shard_map over axis 0)."""
    if "exec" in _CACHE:
        return _CACHE["exec"]
    import jax
    from jax.sharding import Mesh, PartitionSpec, NamedSharding
    from jax.experimental.shard_map import shard_map
    from concourse import bass2jax

    Bl = B_FULL // N_CORES
    nc = build_program(Bl)
    bass2jax.install_neuronx_cc_hook()

    partition_name = nc.partition_id_tensor.name if nc.partition_id_tensor else None
    in_names, out_names, out_avals, zero_shapes = [], [], [], []
    for alloc in nc.m.functions[0].allocations:
        if not isinstance(alloc, mybir.MemoryLocationSet):
            continue
        name = alloc.memorylocations[0].name
        if alloc.kind == "ExternalInput":
            if name != partition_name:
                in_names.append(name)
        elif alloc.kind == "ExternalOutput":
            np_dtype = mybir.dt.np(alloc.dtype)
            out_avals.append(jax.core.ShapedArray(tuple(alloc.tensor_shape), np_dtype))
            out_names.append(name)
            zero_shapes.append((tuple(alloc.tensor_shape), np_dtype))
    n_params = len(in_names)
    all_names = in_names + out_names
    if partition_name is not None:
        all_names.append(partition_name)

    def _body(*args):
        operands = list(args)
        if partition_name is not None:
            operands.append(bass2jax.partition_id_tensor())
        outs = bass2jax._bass_exec_p.bind(
            *operands,
            out_avals=tuple(out_avals),
            in_names=tuple(all_names),
            out_names=tuple(out_names),
            lowering_input_output_aliases=(),
            sim_require_finite=True,
            sim_require_nnan=True,
            nc=nc,
        )
        return tuple(outs)

    devices = jax.devices()[:N_CORES]
    mesh = Mesh(np.asarray(devices), ("core",))
    spec = PartitionSpec("core")
    sharded = jax.jit(
        shard_map(
            _body,
            mesh=mesh,
            in_specs=(spec,) * (n_params + len(out_names)),
            out_specs=(spec,) * len(out_names),
            check_rep=False,
        ),
        keep_unused=True,
    )
    sharding = NamedSharding(mesh, spec)
    _CACHE["exec"] = dict(
        nc=nc, fn=sharded, in_names=in_names, out_names=out_names,
        zero_shapes=zero_shapes, sharding=sharding,
    )
    return _CACHE["exec"]


def _device_args(in_maps):
    """Concat per-core input maps along axis 0 and device_put with sharding."""
    import jax
    ex = _get_exec()
    args = []
    for name in ex["in_names"]:
        arr = np.concatenate([m[name] for m in in_maps], axis=0)
        args.append(arr)
    for shape, dt in ex["zero_shapes"]:
        args.append(np.zeros((N_CORES * shape[0],) + shape[1:], dt))
    return [jax.device_put(a, ex["sharding"]) for a in args]


def kernel(**inputs):
    ex = _get_exec()
    in_maps = host_prep(inputs)
    dargs = _device_args(in_maps)
    outs = ex["fn"](*dargs)
    attn = np.asarray(outs[ex["out_names"].index("attn_o")]).astype(np.float32)
    lp = np.asarray(outs[ex["out_names"].index("lp_o")]).astype(np.float32)
    attn = attn.reshape(B_FULL, 1, T1, T2)
    lp = lp.reshape(B_FULL, 1, T1, T2)
    return attn, lp


def bench(inputs, warmup=2, n_small=16, n_big=64):
    """Marginal per-execution time: (t(n_big) - t(n_small)) / (n_big - n_small),
    which cancels the fixed axon dispatch overhead."""
    import time
    import jax
    ex = _get_exec()
    in_maps = host_prep(inputs)
    dargs = _device_args(in_maps)
    for _ in range(warmup):
        jax.block_until_ready(ex["fn"](*dargs))
    t0 = time.perf_counter()
    out = ex["fn"](*dargs)
    jax.block_until_ready(out)
    t_single = time.perf_counter() - t0

    def burst(n):
        t0 = time.perf_counter()
        outs = [ex["fn"](*dargs) for _ in range(n)]
        jax.block_until_ready(outs)
        return time.perf_counter() - t0

    burst(4)
    margs = []
    for _ in range(3):
        ts = burst(n_small)
        tb = burst(n_big)
        margs.append((tb - ts) / (n_big - n_small))
    t_marg = min(margs)
    return t_single, t_marg
